# revision 15
# baseline (speedup 1.0000x reference)
"""Trainium2 Bass kernel for nn_Cluster_46574625358249 (vq_codebook).

Sharding: 4 fold-regions x 2 k-column-halves = 8 cores (each core owns half
the region's 3456 feature-map pixels and ALL of the region's centers, so the
per-pixel argmax over centers needs no cross-core comms; host sums the two
half-aggregations per region).

Host ships, per core (hi/lo is an exact fp16 split, x ~= hi+lo to ~2^-22):
  fA [128,K_PAD] f16: rows 0-63 hi(feat),  rows 64-127 0
  fB [128,K_PAD] f16: rows 0-63 lo(feat),  rows 64-127 hi(feat)
  cR [128,M_PAD] f16: rows 0-63 hi(cnhat), rows 64-127 lo(cnhat); col cnt is
     the ghost center (reference zero-slot); cols>cnt are exact ghost copies
  vt [128,NKT*65] f32 (+kmask col 64), invx [128,NKT] f32, beta128 [128,1]
Device per k-tile kt (128 pixels, NKT=14):
  PE   az = fA^T@cR + fB^T@cR (two fp16 passes, fp32-grade) -> PSUM
       tiles c0,c1 [512], c23 [1092]
  DVE  TensorTensorReduce on c0,c1: copies PSUM->SBUF f32 (azS) AND chains
       the row max; ACT copies c23 -> SBUF (azY); Pool 2-level pairwise
       tensor_tensor max tree on azY; DVE TTR folds the tree tail into the
       final exact-f32 row max rmax
  onehot wA = (1[az==rmax]-1) in {-1,0} bf16 (host adds back rs):
       DVE is_lt(azS,rmax)*-1; ACT Sign(azS/azY-rmax); Pool is_lt on azY tail
  ACT  colval = sigmoid(rmax*invx+beta); Pool rhsp_kt = bf16(vt*colval)
B2 (transposed aggregation): acc[128m,65] += wA_kt_chunk^T @ rhsp_kt over
  kt; kt 0..7 interleaved into the tail of the B1 loop (keeps the PE p-state
  ramp saturated), kt 8..13 after it; 7 accumulator groups share a PSUM bank
  sequentially so drains batch into [128,455] copies. PE filler matmuls
  during the DMA lead-in burn the 3us p-state ramp at no cost.
Host: agg = aggA+aggB (+ across the core pair) + rs (rs = vt^T @ colval),
out = (agg[:,:64]+value_centers)/(agg[:,64]+1), 64x64 projection, scatter.
"""

import ml_dtypes
import numpy as np

FOLD_H = 2
FOLD_W = 2
SIZE_W = 1296.0
SIZE_H = 384.0
RH, RW = 32, 108          # folded region map H, W
HW = RH * RW              # 3456
K_HALF = HW // 2          # 1728
K_PAD = 1792              # 14*128
NKT = K_PAD // 128        # 14
M_PAD = 2116              # >= max region count (2114) + 1 ghost
MB2 = 2176                # 17*128, wA width for B2 chunking
NCH = MB2 // 128          # 17 B2 m-chunks
R = FOLD_H * FOLD_W
N_CORES = 8

AB_W = 1536               # azAB = m[0:1536), double-buffered (6 banks)
A1_W = 580                # azA1 = m[1536:2116), single-buffered (2 banks)
A1_BIG = 1024             # azA1 tile width; [580:1024) is PE filler scratch
B2SPLIT = 8               # (unused in B2-at-end form)
NFILL = 7                 # pre-loop PE filler matmuls (p-state ramp burn)
NFILL_KT = 4              # per-kt fillers bridging the azA1 WAR window

_BUILT = {}
_LAST_IN_MAPS = None


def _build():
    from concourse import bacc, mybir
    from concourse.tile import TileContext

    f32 = mybir.dt.float32
    f16 = mybir.dt.float16
    bf16 = mybir.dt.bfloat16

    nc = bacc.Bacc(None, target_bir_lowering=False)
    fa_d = nc.dram_tensor("fA", [128, K_PAD], f16, kind="ExternalInput")
    fb_d = nc.dram_tensor("fB", [128, K_PAD], f16, kind="ExternalInput")
    cr_d = nc.dram_tensor("cR", [128, M_PAD], f16, kind="ExternalInput")
    vt_d = nc.dram_tensor("vt", [128, NKT * 65], f32, kind="ExternalInput")
    invx_d = nc.dram_tensor("invx", [128, NKT], f32, kind="ExternalInput")
    beta_d = nc.dram_tensor("beta128", [128, 1], f32, kind="ExternalInput")
    agg_d = nc.dram_tensor("agg", [128, NCH * 65], f32, kind="ExternalOutput")
    cv_d = nc.dram_tensor("cv", [128, NKT], f32, kind="ExternalOutput")

    Sig = mybir.ActivationFunctionType.Sigmoid
    Sgn = mybir.ActivationFunctionType.Sign
    X = mybir.AxisListType.X
    MAX = mybir.AluOpType.max
    MULT = mybir.AluOpType.mult

    with TileContext(nc) as tc:
        with tc.tile_pool(name="big", bufs=1) as big:
            fAh = big.tile([128, 128], f16)              # kt0's lhsT
            fBh = big.tile([128, 128], f16)
            fA = big.tile([128, K_PAD], f16)
            fB = big.tile([128, K_PAD], f16)
            cRa = big.tile([128, AB_W], f16)
            cRb = big.tile([128, A1_W], f16)
            vt = big.tile([128, NKT * 65], f32)
            invx = big.tile([128, NKT], f32)
            beta = big.tile([128, 1], f32)
            wA = big.tile([128, NKT * MB2], bf16)
            rhsp = big.tile([128, NKT * 65], bf16)
            cvall = big.tile([128, NKT], f32)
            aggsb = big.tile([128, NCH * 65], f32)
            filler = big.tile([128, 444], bf16)

            # ---- input DMAs (critical kt0/cR first on fast queues) ----
            nc.sync.dma_start(out=fAh[:], in_=fa_d[:, :128])
            nc.sync.dma_start(out=fBh[:], in_=fb_d[:, :128])
            nc.sync.dma_start(out=cRa[:], in_=cr_d[:, 0:AB_W])
            nc.sync.dma_start(out=cRb[:], in_=cr_d[:, AB_W:M_PAD])
            nc.gpsimd.dma_start(out=fA[:, 128:K_PAD],
                                in_=fa_d[:, 128:K_PAD])
            nc.gpsimd.dma_start(out=fB[:, 128:K_PAD],
                                in_=fb_d[:, 128:K_PAD])
            nc.scalar.dma_start(out=vt[:], in_=vt_d[:, :])
            nc.scalar.dma_start(out=invx[:], in_=invx_d[:, :])
            nc.scalar.dma_start(out=beta[:], in_=beta_d[:, :])

            # wA pad columns [2116:2176) per kt: one strided memset
            wA3 = wA[:].rearrange("p (t m) -> p t m", t=NKT)
            nc.vector.memset(wA3[:, :, M_PAD:MB2], 0.0)
            nc.vector.memset(filler[:], 0.0)

            with tc.tile_pool(name="sm", bufs=4) as smp, \
                 tc.tile_pool(name="pab", bufs=2, space="PSUM") as pab, \
                 tc.tile_pool(name="pa1", bufs=1, space="PSUM") as pa1:

                # azA1 tile: az in [0:580), PE filler scratch in [580:1024)
                # (disjoint subtile ranges -> fillers dodge the WAR on the
                # single-buffered az region and keep the PE p-state ramp hot)
                a1t = pa1.tile([128, A1_BIG], f32, tag="a1")

                def fill(n):
                    for _ in range(n):
                        nc.tensor.matmul(out=a1t[:, A1_W:A1_BIG],
                                         lhsT=filler[:, 0:128],
                                         rhs=filler[:], start=True, stop=True)

                fill(NFILL)   # p-state ramp burn during the DMA lead-in

                for kt in range(NKT):
                    lA = fAh[:] if kt == 0 else fA[:, kt * 128:(kt + 1) * 128]
                    lB = fBh[:] if kt == 0 else fB[:, kt * 128:(kt + 1) * 128]

                    # ---- B1: azAB (double-buffered), two fp16 passes ----
                    ab = pab.tile([128, AB_W], f32, tag="ab")
                    for lo, hi in ((0, 512), (512, 1024), (1024, AB_W)):
                        nc.tensor.matmul(out=ab[:, lo:hi], lhsT=lA,
                                         rhs=cRa[:, lo:hi],
                                         start=True, stop=False)
                        nc.tensor.matmul(out=ab[:, lo:hi], lhsT=lB,
                                         rhs=cRa[:, lo:hi],
                                         start=False, stop=True)
                    # fillers bridge the wait for azA1's WAR release
                    fill(NFILL_KT)
                    # ---- azA1 (single-buffered; the serialization window:
                    # its Sign is ordered first after rmax) ----
                    for lo, hi in ((0, 512), (512, A1_W)):
                        nc.tensor.matmul(out=a1t[:, lo:hi], lhsT=lA,
                                         rhs=cRb[:, lo:hi],
                                         start=True, stop=False)
                        nc.tensor.matmul(out=a1t[:, lo:hi], lhsT=lB,
                                         rhs=cRb[:, lo:hi],
                                         start=False, stop=True)

                    # ---- exact row max (DVE) + combine (Pool) ----
                    mab = smp.tile([128, 1], f32, tag="mab")
                    nc.vector.reduce_max(out=mab[:], in_=ab[:], axis=X)
                    ma1 = smp.tile([128, 1], f32, tag="ma1")
                    nc.vector.reduce_max(out=ma1[:], in_=a1t[:, 0:A1_W],
                                         axis=X)
                    rmax = smp.tile([128, 1], f32, tag="rmax")
                    nc.gpsimd.tensor_scalar(out=rmax[:], in0=mab[:],
                                            scalar1=ma1[:], scalar2=None,
                                            op0=MAX)

                    # colval = sigmoid(rmax*invx + beta); nrm = -rmax
                    nc.scalar.activation(out=cvall[:, kt:kt + 1], in_=rmax[:],
                                         func=Sig, bias=beta[:],
                                         scale=invx[:, kt:kt + 1])
                    nrm = smp.tile([128, 1], f32, tag="nrm")
                    nc.gpsimd.tensor_scalar(out=nrm[:], in0=rmax[:],
                                            scalar1=-1.0, scalar2=None,
                                            op0=MULT)
                    nc.gpsimd.tensor_scalar(
                        out=rhsp[:, kt * 65:(kt + 1) * 65],
                        in0=vt[:, kt * 65:(kt + 1) * 65],
                        scalar1=cvall[:, kt:kt + 1], scalar2=None, op0=MULT)

                    # ---- one-hot (-1/0): ACT Sign, azA1 range FIRST ----
                    wk = kt * MB2
                    nc.scalar.activation(out=wA[:, wk + AB_W:wk + M_PAD],
                                         in_=a1t[:, 0:A1_W], func=Sgn,
                                         bias=nrm[:])
                    nc.scalar.activation(out=wA[:, wk:wk + AB_W],
                                         in_=ab[:], func=Sgn, bias=nrm[:])

            # ---- B2: transposed aggregation GEMM at the end (az pools
            # freed above; PE re-ramps through its first ~3us) ----
            with tc.tile_pool(name="pacc", bufs=4, space="PSUM") as paccp:
                for ci in range(NCH):
                    acc = paccp.tile([128, 65], f32, tag="acc")
                    for k2 in range(NKT):
                        w2 = k2 * MB2 + ci * 128
                        nc.tensor.matmul(
                            out=acc[:],
                            lhsT=wA[:, w2:w2 + 128],
                            rhs=rhsp[:, k2 * 65:(k2 + 1) * 65],
                            start=(k2 == 0), stop=(k2 == NKT - 1))
                    if ci % 2 == 0:
                        nc.vector.tensor_copy(
                            out=aggsb[:, ci * 65:(ci + 1) * 65], in_=acc[:])
                    else:
                        nc.scalar.copy(
                            out=aggsb[:, ci * 65:(ci + 1) * 65], in_=acc[:])
            nc.sync.dma_start(out=agg_d[:, :], in_=aggsb[:])
            nc.sync.dma_start(out=cv_d[:, :], in_=cvall[:])
    nc.compile()
    return nc


def _f32(x):
    return np.ascontiguousarray(np.asarray(x), dtype=np.float32)


def _region_indices(points):
    rh = np.float32(SIZE_H / FOLD_H)
    rw = np.float32(SIZE_W / FOLD_W)
    px, py = points[:, 0], points[:, 1]
    idxs = []
    for i in range(FOLD_H):
        for j in range(FOLD_W):
            m = (py > rh * i) & (py <= rh * (i + 1)) & \
                (px > rw * j) & (px <= rw * (j + 1))
            idxs.append(np.nonzero(m)[0])
    return idxs


def _bilinear_taps(pts):
    one = np.float32(1.0)
    gridx = pts[:, 0] / np.float32(SIZE_W - 1.0) * np.float32(2.0) - one
    gridy = pts[:, 1] / np.float32(SIZE_H - 1.0) * np.float32(2.0) - one
    gx = (gridx + one) * np.float32(RW * 0.5) - np.float32(0.5)
    gy = (gridy + one) * np.float32(RH * 0.5) - np.float32(0.5)
    x0 = np.floor(gx)
    y0 = np.floor(gy)
    wx = (gx - x0).astype(np.float32)
    wy = (gy - y0).astype(np.float32)
    x0i = np.clip(x0, 0, RW - 1).astype(np.int32)
    x1i = np.clip(x0 + 1.0, 0, RW - 1).astype(np.int32)
    y0i = np.clip(y0, 0, RH - 1).astype(np.int32)
    y1i = np.clip(y0 + 1.0, 0, RH - 1).astype(np.int32)
    taps = np.stack([y0i * RW + x0i, y0i * RW + x1i,
                     y1i * RW + x0i, y1i * RW + x1i], axis=1)
    w = np.stack([(one - wx) * (one - wy), wx * (one - wy),
                  (one - wx) * wy, wx * wy], axis=1).astype(np.float32)
    # Clamp-collapsed points (all 4 taps at one pixel, e.g. ghost slots and
    # border points): weight (1,0,0,0) makes those columns bit-identical to
    # the ghost column, so argmax ties are exact and deterministic.
    collapsed = (x0i == x1i) & (y0i == y1i)
    w[collapsed] = np.array([1.0, 0.0, 0.0, 0.0], np.float32)
    return taps, w


def _hilo(x):
    hi = x.astype(np.float16)
    lo = (x - hi.astype(np.float32)).astype(np.float16)
    return hi, lo


def kernel(points, x, W_f, b_f, W_v, b_v, W_proj, b_proj, sim_alpha, sim_beta):
    from concourse.bass_utils import run_bass_kernel_spmd

    points = _f32(points)[0]
    x = _f32(x)[0]
    W_f, b_f = _f32(W_f), _f32(b_f)
    W_v, b_v = _f32(W_v), _f32(b_v)
    W_proj, b_proj = _f32(W_proj), _f32(b_proj)
    alpha = _f32(sim_alpha).reshape(-1)[0]
    beta = _f32(sim_beta).reshape(-1)[0]
    N = points.shape[0]

    idxs = _region_indices(points)
    cnts = [len(ix) for ix in idxs]
    assert max(cnts) + 1 <= M_PAD, cnts

    Wfb = np.concatenate([W_f.T, b_f[None, :]], axis=0).astype(np.float32)
    Wvb = np.concatenate([W_v.T, b_v[None, :]], axis=0).astype(np.float32)
    beta128 = np.full((128, 1), beta, np.float32)

    in_maps = []
    vcts = []
    vts_host = []
    for r in range(R):
        i, j = divmod(r, FOLD_W)
        xr = x[:, i * RH:(i + 1) * RH, j * RW:(j + 1) * RW].reshape(64, HW)
        idx_r = idxs[r]
        cnt = len(idx_r)
        pts_r = np.zeros((cnt + 1, 2), np.float32)
        pts_r[:cnt] = points[idx_r]
        taps, w = _bilinear_taps(pts_r)
        g = xr[:, taps]                                    # [64, cnt+1, 4]
        xg = np.einsum("cmt,mt->cm", g, w).astype(np.float32)
        xg1 = np.ascontiguousarray(
            np.concatenate([xg, np.ones((1, cnt + 1), np.float32)], axis=0))
        # centers + l2 scale (alpha folded in) -> scaled center features
        centers = (xg1.T @ Wfb).astype(np.float32)         # [cnt+1, 64]
        nc2 = (centers * centers).sum(axis=1, dtype=np.float32)
        s = ((np.float32(1.0) / np.sqrt(nc2 + np.float32(1e-12))) * alpha
             ).astype(np.float32)
        cnhatT = (centers * s[:, None]).T                  # [64, cnt+1]
        chi, clo = _hilo(cnhatT)
        cR = np.empty((128, M_PAD), np.float16)
        cR[0:64, :cnt + 1] = chi
        cR[64:128, :cnt + 1] = clo
        cR[0:64, cnt + 1:] = chi[:, cnt:cnt + 1]           # ghost copies
        cR[64:128, cnt + 1:] = clo[:, cnt:cnt + 1]
        # value centers (host side of the output combine)
        vcT = np.ascontiguousarray((xg1.T @ Wvb).T)        # [64, cnt+1]
        vcts.append(vcT)
        # full-map features for sim columns + alpha/|feat| scales
        xr1 = np.concatenate([xr, np.ones((1, HW), np.float32)], axis=0)
        featT = (xr1.T @ Wfb).astype(np.float32)           # [HW, 64]
        nfx = (featT * featT).sum(axis=1, dtype=np.float32)
        invx_full = (np.float32(1.0) / np.sqrt(nfx + np.float32(1e-12))
                     ).astype(np.float32)
        # per-column values (incl bias row + kmask col)
        vt_full = (xr1.T @ np.concatenate(
            [Wvb, np.zeros((65, 1), np.float32)], axis=1)).astype(np.float32)
        vt_full[:, 64] = 1.0                               # kmask for real k
        for h in range(2):
            fh = np.zeros((64, K_PAD), np.float32)
            fh[:, :K_HALF] = featT[h * K_HALF:(h + 1) * K_HALF].T
            fhi, flo = _hilo(fh)
            z = np.zeros_like(fhi)
            fAm = np.ascontiguousarray(np.concatenate([fhi, z], axis=0))
            fBm = np.ascontiguousarray(np.concatenate([flo, fhi], axis=0))
            vt_np = np.zeros((K_PAD, 65), np.float32)
            vt_np[:K_HALF] = vt_full[h * K_HALF:(h + 1) * K_HALF, :65]
            vt_in = np.ascontiguousarray(
                vt_np.reshape(NKT, 128, 65).transpose(1, 0, 2).reshape(
                    128, NKT * 65))
            iv = np.ones((K_PAD,), np.float32)
            iv[:K_HALF] = invx_full[h * K_HALF:(h + 1) * K_HALF]
            invx = np.ascontiguousarray(iv.reshape(NKT, 128).T)  # [128, NKT]
            vts_host.append(vt_np)
            in_maps.append({
                "fA": fAm, "fB": fBm, "cR": cR, "vt": vt_in,
                "invx": invx, "beta128": beta128,
            })

    global _LAST_IN_MAPS
    _LAST_IN_MAPS = in_maps
    if 0 not in _BUILT:
        _BUILT[0] = _build()
    res = run_bass_kernel_spmd(_BUILT[0], in_maps,
                               core_ids=list(range(N_CORES)))
    results = res.results

    out = np.zeros((64, N), np.float32)
    for r in range(R):
        agg = None
        rs = np.zeros((65,), np.float32)
        for h in range(2):
            core = 2 * r + h
            rr = results[core]
            # agg[m, c] with m = ci*128 + p
            a = rr["agg"].reshape(
                128, NCH, 65).transpose(1, 0, 2).reshape(NCH * 128, 65)
            agg = a if agg is None else agg + a
            # rs[c] = sum_k rhsp[k, c] with the device's bf16 rounding
            # (B2's -1 offset uses bf16 rhsp, so the correction must too)
            cv = rr["cv"].T.reshape(K_PAD)           # [NKT,128] -> k order
            rhs_bf = (vts_host[core] * cv[:, None]).astype(
                ml_dtypes.bfloat16).astype(np.float32)
            rs += rhs_bf.sum(axis=0)
        idx_r = idxs[r]
        cnt = len(idx_r)
        vcT = vcts[r]
        a = agg[:cnt] + rs[None, :]
        ort = (a[:, :64].T + vcT[:, :cnt]) / \
            (a[:, 64] + np.float32(1.0))[None, :]
        proj = W_proj @ ort + b_proj[:, None]
        mask = np.any(ort != 0.0, axis=0)
        out[:, idx_r] = proj * mask[None, :]
    return out[None, :, None, :]


# revision 21
# speedup vs baseline: 1.0981x; 1.0981x over previous
"""Trainium2 Bass kernel for nn_Cluster_46574625358249 (vq_codebook).

Sharding: 4 fold-regions x 2 k-column-halves = 8 cores (each core owns half
the region's 3456 feature-map pixels and ALL of the region's centers, so the
per-pixel argmax over centers needs no cross-core comms; host sums the two
half-aggregations per region).

Host ships, per core (hi/lo is an exact fp16 split, x ~= hi+lo to ~2^-22):
  fA [128,K_PAD] f16: rows 0-63 hi(feat),  rows 64-127 0
  fB [128,K_PAD] f16: rows 0-63 lo(feat),  rows 64-127 hi(feat)
  cR [128,M_PAD] f16: rows 0-63 hi(cnhat), rows 64-127 lo(cnhat); col cnt is
     the ghost center (reference zero-slot); cols>cnt are exact ghost copies
  vt [128,NKT*65] f32 (+kmask col 64), invx [128,NKT] f32, beta128 [128,1]
Device per k-tile kt (128 pixels, NKT=14):
  PE   az = fA^T@cR + fB^T@cR (two fp16 passes, fp32-grade) -> PSUM
       tiles c0,c1 [512], c23 [1092]
  DVE  TensorTensorReduce on c0,c1: copies PSUM->SBUF f32 (azS) AND chains
       the row max; ACT copies c23 -> SBUF (azY); Pool 2-level pairwise
       tensor_tensor max tree on azY; DVE TTR folds the tree tail into the
       final exact-f32 row max rmax
  onehot wA = (1[az==rmax]-1) in {-1,0} bf16 (host adds back rs):
       DVE is_lt(azS,rmax)*-1; ACT Sign(azS/azY-rmax); Pool is_lt on azY tail
  ACT  colval = sigmoid(rmax*invx+beta); Pool rhsp_kt = bf16(vt*colval)
B2 (transposed aggregation): acc[128m,65] += wA_kt_chunk^T @ rhsp_kt over
  kt; kt 0..7 interleaved into the tail of the B1 loop (keeps the PE p-state
  ramp saturated), kt 8..13 after it; 7 accumulator groups share a PSUM bank
  sequentially so drains batch into [128,455] copies. PE filler matmuls
  during the DMA lead-in burn the 3us p-state ramp at no cost.
Host: agg = aggA+aggB (+ across the core pair) + rs (rs = vt^T @ colval),
out = (agg[:,:64]+value_centers)/(agg[:,64]+1), 64x64 projection, scatter.
"""

import ml_dtypes
import numpy as np

FOLD_H = 2
FOLD_W = 2
SIZE_W = 1296.0
SIZE_H = 384.0
RH, RW = 32, 108          # folded region map H, W
HW = RH * RW              # 3456
K_HALF = HW // 2          # 1728
K_PAD = 1792              # 14*128
NKT = K_PAD // 128        # 14
M_PAD = 2116              # >= max region count (2114) + 1 ghost
MB2 = 2176                # 17*128, wA width for B2 chunking
NCH = MB2 // 128          # 17 B2 m-chunks
R = FOLD_H * FOLD_W
N_CORES = 8

A_W = 1024                # azA = m[0:1024), double-buffered (4 banks)
B_W = 512                 # azB = m[1024:1536), double-buffered (2 banks)
AB_W = A_W + B_W          # 1536
A1_W = 580                # azA1 = m[1536:2116), single-buffered (2 banks)
A1_BIG = 1024             # azA1 tile width; [580:1024) is PE filler scratch
B2SPLIT = 8               # (unused in B2-at-end form)
NFILL = 10                # pre-loop PE filler matmuls (p-state ramp burn)
NFILL_KT = 0              # per-kt fillers (off: they joined the WAR chain)

_BUILT = {}
_LAST_IN_MAPS = None


def _build():
    from concourse import bacc, mybir
    from concourse.tile import TileContext

    f32 = mybir.dt.float32
    f16 = mybir.dt.float16
    bf16 = mybir.dt.bfloat16

    nc = bacc.Bacc(None, target_bir_lowering=False)
    fa_d = nc.dram_tensor("fA", [128, K_PAD], f16, kind="ExternalInput")
    fb_d = nc.dram_tensor("fB", [128, K_PAD], f16, kind="ExternalInput")
    cr_d = nc.dram_tensor("cR", [128, M_PAD], f16, kind="ExternalInput")
    vt_d = nc.dram_tensor("vt", [128, NKT * 65], f32, kind="ExternalInput")
    invx_d = nc.dram_tensor("invx", [128, NKT], f32, kind="ExternalInput")
    beta_d = nc.dram_tensor("beta128", [128, 1], f32, kind="ExternalInput")
    agg_d = nc.dram_tensor("agg", [128, NCH * 65], f32, kind="ExternalOutput")
    cv_d = nc.dram_tensor("cv", [128, NKT], f32, kind="ExternalOutput")

    Sig = mybir.ActivationFunctionType.Sigmoid
    Sgn = mybir.ActivationFunctionType.Sign
    X = mybir.AxisListType.X
    MAX = mybir.AluOpType.max
    MULT = mybir.AluOpType.mult

    with TileContext(nc) as tc:
        with tc.tile_pool(name="big", bufs=1) as big:
            fAh = big.tile([128, 128], f16)              # kt0's lhsT
            fBh = big.tile([128, 128], f16)
            fA = big.tile([128, K_PAD], f16)
            fB = big.tile([128, K_PAD], f16)
            cRa = big.tile([128, AB_W], f16)
            cRb = big.tile([128, A1_W], f16)
            vt = big.tile([128, NKT * 65], f32)
            invx = big.tile([128, NKT], f32)
            beta = big.tile([128, 1], f32)
            wA = big.tile([128, NKT * MB2], bf16)
            rhsp = big.tile([128, NKT * 65], bf16)
            cvall = big.tile([128, NKT], f32)
            aggsb = big.tile([128, NCH * 65], f32)
            filler = big.tile([128, 444], bf16)

            # ---- input DMAs (critical kt0/cR first on fast queues) ----
            nc.sync.dma_start(out=fAh[:], in_=fa_d[:, :128])
            nc.sync.dma_start(out=fBh[:], in_=fb_d[:, :128])
            nc.sync.dma_start(out=cRa[:], in_=cr_d[:, 0:AB_W])
            nc.sync.dma_start(out=cRb[:], in_=cr_d[:, AB_W:M_PAD])
            nc.gpsimd.dma_start(out=fA[:, 128:K_PAD],
                                in_=fa_d[:, 128:K_PAD])
            nc.gpsimd.dma_start(out=fB[:, 128:K_PAD],
                                in_=fb_d[:, 128:K_PAD])
            nc.scalar.dma_start(out=vt[:], in_=vt_d[:, :])
            nc.scalar.dma_start(out=invx[:], in_=invx_d[:, :])
            nc.scalar.dma_start(out=beta[:], in_=beta_d[:, :])

            # wA pad columns [2116:2176) per kt: one strided memset
            wA3 = wA[:].rearrange("p (t m) -> p t m", t=NKT)
            nc.vector.memset(wA3[:, :, M_PAD:MB2], 0.0)
            nc.vector.memset(filler[:], 0.0)

            with tc.tile_pool(name="sm", bufs=4) as smp, \
                 tc.tile_pool(name="pa", bufs=2, space="PSUM") as pa, \
                 tc.tile_pool(name="pb", bufs=2, space="PSUM") as pb, \
                 tc.tile_pool(name="pa1", bufs=1, space="PSUM") as pa1:

                # azA1 tile: az in [0:580), PE filler scratch in [580:1024)
                # (disjoint subtile ranges -> fillers dodge the WAR on the
                # single-buffered az region and keep the PE p-state ramp hot)
                a1t = pa1.tile([128, A1_BIG], f32, tag="a1")

                def fill(n):
                    for _ in range(n):
                        nc.tensor.matmul(out=a1t[:, A1_W:A1_BIG],
                                         lhsT=filler[:, 0:128],
                                         rhs=filler[:], start=True, stop=True)

                fill(NFILL)   # p-state ramp burn during the DMA lead-in

                for kt in range(NKT):
                    lA = fAh[:] if kt == 0 else fA[:, kt * 128:(kt + 1) * 128]
                    lB = fBh[:] if kt == 0 else fB[:, kt * 128:(kt + 1) * 128]

                    # ---- B1: azA, azB (double-buffered), 2 fp16 passes
                    aa = pa.tile([128, A_W], f32, tag="aa")
                    for lo, hi in ((0, 512), (512, A_W)):
                        nc.tensor.matmul(out=aa[:, lo:hi], lhsT=lA,
                                         rhs=cRa[:, lo:hi],
                                         start=True, stop=False)
                        nc.tensor.matmul(out=aa[:, lo:hi], lhsT=lB,
                                         rhs=cRa[:, lo:hi],
                                         start=False, stop=True)
                    bb = pb.tile([128, B_W], f32, tag="bb")
                    nc.tensor.matmul(out=bb[:], lhsT=lA,
                                     rhs=cRa[:, A_W:AB_W],
                                     start=True, stop=False)
                    nc.tensor.matmul(out=bb[:], lhsT=lB,
                                     rhs=cRa[:, A_W:AB_W],
                                     start=False, stop=True)
                    # fillers bridge the wait for azA1's WAR release
                    fill(NFILL_KT)
                    # ---- azA1 (single-buffered; the serialization window:
                    # its Sign is ordered first after rmax) ----
                    for lo, hi in ((0, 512), (512, A1_W)):
                        nc.tensor.matmul(out=a1t[:, lo:hi], lhsT=lA,
                                         rhs=cRb[:, lo:hi],
                                         start=True, stop=False)
                        nc.tensor.matmul(out=a1t[:, lo:hi], lhsT=lB,
                                         rhs=cRb[:, lo:hi],
                                         start=False, stop=True)

                    # ---- exact row max (DVE) + combine (Pool) ----
                    maa = smp.tile([128, 1], f32, tag="maa")
                    nc.vector.reduce_max(out=maa[:], in_=aa[:], axis=X)
                    mbb = smp.tile([128, 1], f32, tag="mbb")
                    nc.vector.reduce_max(out=mbb[:], in_=bb[:], axis=X)
                    mab = smp.tile([128, 1], f32, tag="mab")
                    nc.gpsimd.tensor_scalar(out=mab[:], in0=maa[:],
                                            scalar1=mbb[:], scalar2=None,
                                            op0=MAX)
                    ma1 = smp.tile([128, 1], f32, tag="ma1")
                    nc.vector.reduce_max(out=ma1[:], in_=a1t[:, 0:A1_W],
                                         axis=X)
                    wk = kt * MB2
                    rmax = smp.tile([128, 1], f32, tag="rmax")
                    nrm = smp.tile([128, 1], f32, tag="nrm")
                    nc.gpsimd.tensor_scalar(out=rmax[:], in0=mab[:],
                                            scalar1=ma1[:],
                                            scalar2=None, op0=MAX)
                    nc.gpsimd.tensor_scalar(out=nrm[:], in0=rmax[:],
                                            scalar1=-1.0,
                                            scalar2=None, op0=MULT)
                    nc.scalar.activation(out=wA[:, wk + AB_W:wk + M_PAD],
                                         in_=a1t[:, 0:A1_W], func=Sgn,
                                         bias=nrm[:])
                    nc.scalar.activation(out=cvall[:, kt:kt + 1],
                                         in_=rmax[:], func=Sig,
                                         bias=beta[:],
                                         scale=invx[:, kt:kt + 1])
                    nc.scalar.activation(out=wA[:, wk:wk + A_W],
                                         in_=aa[:], func=Sgn, bias=nrm[:])
                    nc.scalar.activation(out=wA[:, wk + A_W:wk + AB_W],
                                         in_=bb[:], func=Sgn, bias=nrm[:])
                    nc.gpsimd.tensor_scalar(
                        out=rhsp[:, kt * 65:(kt + 1) * 65],
                        in0=vt[:, kt * 65:(kt + 1) * 65],
                        scalar1=cvall[:, kt:kt + 1], scalar2=None, op0=MULT)

            # ---- B2: transposed aggregation GEMM at the end (az pools
            # freed above; PE re-ramps through its first ~3us) ----
            with tc.tile_pool(name="pacc", bufs=1, space="PSUM") as paccp:
                # 7 sequential accumulator groups per PSUM bank (each group
                # closes before the next opens at a different offset), so
                # drains batch into [128, 455] copies; 4 banks round-robin
                acc = paccp.tile([128, 2048], f32, tag="acc")
                for ci in range(NCH):
                    bank = (ci // 7) % 4
                    off = bank * 512 + (ci % 7) * 65
                    for k2 in range(NKT):
                        w2 = k2 * MB2 + ci * 128
                        nc.tensor.matmul(
                            out=acc[:, off:off + 65],
                            lhsT=wA[:, w2:w2 + 128],
                            rhs=rhsp[:, k2 * 65:(k2 + 1) * 65],
                            start=(k2 == 0), stop=(k2 == NKT - 1))
                    if ci % 7 == 6 or ci == NCH - 1:
                        lo_ci = ci - (ci % 7)
                        w = (ci % 7 + 1) * 65
                        if (ci // 7) % 2 == 0:
                            nc.vector.tensor_copy(
                                out=aggsb[:, lo_ci * 65:lo_ci * 65 + w],
                                in_=acc[:, bank * 512:bank * 512 + w])
                        else:
                            nc.scalar.copy(
                                out=aggsb[:, lo_ci * 65:lo_ci * 65 + w],
                                in_=acc[:, bank * 512:bank * 512 + w])
            nc.sync.dma_start(out=cv_d[:, :], in_=cvall[:])
            for lo, hi in ((0, 6 * 65), (6 * 65, 12 * 65),
                           (12 * 65, NCH * 65)):
                nc.sync.dma_start(out=agg_d[:, lo:hi], in_=aggsb[:, lo:hi])
    nc.compile()
    return nc


def _f32(x):
    return np.ascontiguousarray(np.asarray(x), dtype=np.float32)


def _region_indices(points):
    rh = np.float32(SIZE_H / FOLD_H)
    rw = np.float32(SIZE_W / FOLD_W)
    px, py = points[:, 0], points[:, 1]
    idxs = []
    for i in range(FOLD_H):
        for j in range(FOLD_W):
            m = (py > rh * i) & (py <= rh * (i + 1)) & \
                (px > rw * j) & (px <= rw * (j + 1))
            idxs.append(np.nonzero(m)[0])
    return idxs


def _bilinear_taps(pts):
    one = np.float32(1.0)
    gridx = pts[:, 0] / np.float32(SIZE_W - 1.0) * np.float32(2.0) - one
    gridy = pts[:, 1] / np.float32(SIZE_H - 1.0) * np.float32(2.0) - one
    gx = (gridx + one) * np.float32(RW * 0.5) - np.float32(0.5)
    gy = (gridy + one) * np.float32(RH * 0.5) - np.float32(0.5)
    x0 = np.floor(gx)
    y0 = np.floor(gy)
    wx = (gx - x0).astype(np.float32)
    wy = (gy - y0).astype(np.float32)
    x0i = np.clip(x0, 0, RW - 1).astype(np.int32)
    x1i = np.clip(x0 + 1.0, 0, RW - 1).astype(np.int32)
    y0i = np.clip(y0, 0, RH - 1).astype(np.int32)
    y1i = np.clip(y0 + 1.0, 0, RH - 1).astype(np.int32)
    taps = np.stack([y0i * RW + x0i, y0i * RW + x1i,
                     y1i * RW + x0i, y1i * RW + x1i], axis=1)
    w = np.stack([(one - wx) * (one - wy), wx * (one - wy),
                  (one - wx) * wy, wx * wy], axis=1).astype(np.float32)
    # Clamp-collapsed points (all 4 taps at one pixel, e.g. ghost slots and
    # border points): weight (1,0,0,0) makes those columns bit-identical to
    # the ghost column, so argmax ties are exact and deterministic.
    collapsed = (x0i == x1i) & (y0i == y1i)
    w[collapsed] = np.array([1.0, 0.0, 0.0, 0.0], np.float32)
    return taps, w


def _hilo(x):
    hi = x.astype(np.float16)
    lo = (x - hi.astype(np.float32)).astype(np.float16)
    return hi, lo


def kernel(points, x, W_f, b_f, W_v, b_v, W_proj, b_proj, sim_alpha, sim_beta):
    from concourse.bass_utils import run_bass_kernel_spmd

    points = _f32(points)[0]
    x = _f32(x)[0]
    W_f, b_f = _f32(W_f), _f32(b_f)
    W_v, b_v = _f32(W_v), _f32(b_v)
    W_proj, b_proj = _f32(W_proj), _f32(b_proj)
    alpha = _f32(sim_alpha).reshape(-1)[0]
    beta = _f32(sim_beta).reshape(-1)[0]
    N = points.shape[0]

    idxs = _region_indices(points)
    cnts = [len(ix) for ix in idxs]
    assert max(cnts) + 1 <= M_PAD, cnts

    Wfb = np.concatenate([W_f.T, b_f[None, :]], axis=0).astype(np.float32)
    Wvb = np.concatenate([W_v.T, b_v[None, :]], axis=0).astype(np.float32)
    beta128 = np.full((128, 1), beta, np.float32)

    in_maps = []
    vcts = []
    vts_host = []
    for r in range(R):
        i, j = divmod(r, FOLD_W)
        xr = x[:, i * RH:(i + 1) * RH, j * RW:(j + 1) * RW].reshape(64, HW)
        idx_r = idxs[r]
        cnt = len(idx_r)
        pts_r = np.zeros((cnt + 1, 2), np.float32)
        pts_r[:cnt] = points[idx_r]
        taps, w = _bilinear_taps(pts_r)
        g = xr[:, taps]                                    # [64, cnt+1, 4]
        xg = np.einsum("cmt,mt->cm", g, w).astype(np.float32)
        xg1 = np.ascontiguousarray(
            np.concatenate([xg, np.ones((1, cnt + 1), np.float32)], axis=0))
        # centers + l2 scale (alpha folded in) -> scaled center features
        centers = (xg1.T @ Wfb).astype(np.float32)         # [cnt+1, 64]
        nc2 = (centers * centers).sum(axis=1, dtype=np.float32)
        s = ((np.float32(1.0) / np.sqrt(nc2 + np.float32(1e-12))) * alpha
             ).astype(np.float32)
        cnhatT = (centers * s[:, None]).T                  # [64, cnt+1]
        chi, clo = _hilo(cnhatT)
        cR = np.empty((128, M_PAD), np.float16)
        cR[0:64, :cnt + 1] = chi
        cR[64:128, :cnt + 1] = clo
        cR[0:64, cnt + 1:] = chi[:, cnt:cnt + 1]           # ghost copies
        cR[64:128, cnt + 1:] = clo[:, cnt:cnt + 1]
        # value centers (host side of the output combine)
        vcT = np.ascontiguousarray((xg1.T @ Wvb).T)        # [64, cnt+1]
        vcts.append(vcT)
        # full-map features for sim columns + alpha/|feat| scales
        xr1 = np.concatenate([xr, np.ones((1, HW), np.float32)], axis=0)
        featT = (xr1.T @ Wfb).astype(np.float32)           # [HW, 64]
        nfx = (featT * featT).sum(axis=1, dtype=np.float32)
        invx_full = (np.float32(1.0) / np.sqrt(nfx + np.float32(1e-12))
                     ).astype(np.float32)
        # per-column values (incl bias row + kmask col)
        vt_full = (xr1.T @ np.concatenate(
            [Wvb, np.zeros((65, 1), np.float32)], axis=1)).astype(np.float32)
        vt_full[:, 64] = 1.0                               # kmask for real k
        for h in range(2):
            fh = np.zeros((64, K_PAD), np.float32)
            fh[:, :K_HALF] = featT[h * K_HALF:(h + 1) * K_HALF].T
            fhi, flo = _hilo(fh)
            z = np.zeros_like(fhi)
            fAm = np.ascontiguousarray(np.concatenate([fhi, z], axis=0))
            fBm = np.ascontiguousarray(np.concatenate([flo, fhi], axis=0))
            vt_np = np.zeros((K_PAD, 65), np.float32)
            vt_np[:K_HALF] = vt_full[h * K_HALF:(h + 1) * K_HALF, :65]
            vt_in = np.ascontiguousarray(
                vt_np.reshape(NKT, 128, 65).transpose(1, 0, 2).reshape(
                    128, NKT * 65))
            iv = np.ones((K_PAD,), np.float32)
            iv[:K_HALF] = invx_full[h * K_HALF:(h + 1) * K_HALF]
            invx = np.ascontiguousarray(iv.reshape(NKT, 128).T)  # [128, NKT]
            vts_host.append(vt_np)
            in_maps.append({
                "fA": fAm, "fB": fBm, "cR": cR, "vt": vt_in,
                "invx": invx, "beta128": beta128,
            })

    global _LAST_IN_MAPS
    _LAST_IN_MAPS = in_maps
    if 0 not in _BUILT:
        _BUILT[0] = _build()
    res = run_bass_kernel_spmd(_BUILT[0], in_maps,
                               core_ids=list(range(N_CORES)))
    results = res.results

    out = np.zeros((64, N), np.float32)
    for r in range(R):
        agg = None
        rs = np.zeros((65,), np.float32)
        for h in range(2):
            core = 2 * r + h
            rr = results[core]
            # agg[m, c] with m = ci*128 + p
            a = rr["agg"].reshape(
                128, NCH, 65).transpose(1, 0, 2).reshape(NCH * 128, 65)
            agg = a if agg is None else agg + a
            # rs[c] = sum_k rhsp[k, c] with the device's bf16 rounding
            # (B2's -1 offset uses bf16 rhsp, so the correction must too)
            cv = rr["cv"].T.reshape(K_PAD)           # [NKT,128] -> k order
            rhs_bf = (vts_host[core] * cv[:, None]).astype(
                ml_dtypes.bfloat16).astype(np.float32)
            rs += rhs_bf.sum(axis=0)
        idx_r = idxs[r]
        cnt = len(idx_r)
        vcT = vcts[r]
        a = agg[:cnt] + rs[None, :]
        ort = (a[:, :64].T + vcT[:, :cnt]) / \
            (a[:, 64] + np.float32(1.0))[None, :]
        proj = W_proj @ ort + b_proj[:, None]
        mask = np.any(ort != 0.0, axis=0)
        out[:, idx_r] = proj * mask[None, :]
    return out[None, :, None, :]


# revision 26
# speedup vs baseline: 1.2009x; 1.0936x over previous
"""Trainium2 Bass kernel for nn_Cluster_46574625358249 (vq_codebook).

Sharding: 4 fold-regions x 2 k-column-halves = 8 cores (each core owns half
the region's 3456 feature-map pixels and ALL of the region's centers, so the
per-pixel argmax over centers needs no cross-core comms; host sums the two
half-aggregations per region).

Host ships, per core (hi/lo is an exact fp16 split, x ~= hi+lo to ~2^-22):
  fA [128,K_PAD] f16: rows 0-63 hi(feat),  rows 64-127 0
  fB [128,K_PAD] f16: rows 0-63 lo(feat),  rows 64-127 hi(feat)
  cR [128,M_PAD] f16: rows 0-63 hi(cnhat), rows 64-127 lo(cnhat); col cnt is
     the ghost center (reference zero-slot); cols>cnt are exact ghost copies
  vt [128,NKT*65] f32 (+kmask col 64), invx [128,NKT] f32, beta128 [128,1]
Device per k-tile kt (128 pixels, NKT=14):
  PE   az = fA^T@cR + fB^T@cR (two fp16 passes, fp32-grade) -> PSUM
       tiles c0,c1 [512], c23 [1092]
  DVE  TensorTensorReduce on c0,c1: copies PSUM->SBUF f32 (azS) AND chains
       the row max; ACT copies c23 -> SBUF (azY); Pool 2-level pairwise
       tensor_tensor max tree on azY; DVE TTR folds the tree tail into the
       final exact-f32 row max rmax
  onehot wA = (1[az==rmax]-1) in {-1,0} bf16 (host adds back rs):
       DVE is_lt(azS,rmax)*-1; ACT Sign(azS/azY-rmax); Pool is_lt on azY tail
  ACT  colval = sigmoid(rmax*invx+beta); Pool rhsp_kt = bf16(vt*colval)
B2 (transposed aggregation): acc[128m,65] += wA_kt_chunk^T @ rhsp_kt over
  kt; kt 0..7 interleaved into the tail of the B1 loop (keeps the PE p-state
  ramp saturated), kt 8..13 after it; 7 accumulator groups share a PSUM bank
  sequentially so drains batch into [128,455] copies. PE filler matmuls
  during the DMA lead-in burn the 3us p-state ramp at no cost.
Host: agg = aggA+aggB (+ across the core pair) + rs (rs = vt^T @ colval),
out = (agg[:,:64]+value_centers)/(agg[:,64]+1), 64x64 projection, scatter.
"""

import ml_dtypes
import numpy as np

FOLD_H = 2
FOLD_W = 2
SIZE_W = 1296.0
SIZE_H = 384.0
RH, RW = 32, 108          # folded region map H, W
HW = RH * RW              # 3456
K_HALF = HW // 2          # 1728
K_PAD = 1792              # 14*128
NKT = K_PAD // 128        # 14
M_PAD = 2116              # >= max region count (2114) + 1 ghost
MB2 = 2176                # 17*128, wA width for B2 chunking
NCH = MB2 // 128          # 17 B2 m-chunks
R = FOLD_H * FOLD_W
N_CORES = 8

A_W = 1024                # azA = m[0:1024), double-buffered (4 banks)
B_W = 512                 # azB = m[1024:1536), double-buffered (2 banks)
AB_W = A_W + B_W          # 1536
A1_W = 580                # azA1 = m[1536:2116), single-buffered (2 banks)
A1_BIG = 1024             # azA1 tile width; [580:1024) is PE filler scratch
B2SPLIT = 8               # (unused in B2-at-end form)
NFILL = 0                # pre-loop PE filler matmuls (p-state ramp burn)
NFILL_KT = 0              # per-kt fillers (off: they joined the WAR chain)

_BUILT = {}
_LAST_IN_MAPS = None


def _build():
    from concourse import bacc, mybir
    from concourse.tile import TileContext

    f32 = mybir.dt.float32
    f16 = mybir.dt.float16
    bf16 = mybir.dt.bfloat16

    nc = bacc.Bacc(None, target_bir_lowering=False)
    fa_d = nc.dram_tensor("fA", [128, K_PAD], f16, kind="ExternalInput")
    fb_d = nc.dram_tensor("fB", [128, K_PAD], f16, kind="ExternalInput")
    cr_d = nc.dram_tensor("cR", [128, M_PAD], f16, kind="ExternalInput")
    vt_d = nc.dram_tensor("vt", [128, NKT * 65], f32, kind="ExternalInput")
    invx_d = nc.dram_tensor("invx", [128, NKT], f32, kind="ExternalInput")
    beta_d = nc.dram_tensor("beta128", [128, 1], f32, kind="ExternalInput")
    agg_d = nc.dram_tensor("agg", [128, NCH * 65], f32, kind="ExternalOutput")
    aggB_d = nc.dram_tensor("aggB", [128, NCH * 65], f32,
                            kind="ExternalOutput")
    cv_d = nc.dram_tensor("cv", [128, NKT], f32, kind="ExternalOutput")

    Sig = mybir.ActivationFunctionType.Sigmoid
    Sgn = mybir.ActivationFunctionType.Sign
    X = mybir.AxisListType.X
    MAX = mybir.AluOpType.max
    MULT = mybir.AluOpType.mult
    IS_LT = mybir.AluOpType.is_lt

    with TileContext(nc) as tc:
        with tc.tile_pool(name="big", bufs=1) as big:
            fAh = big.tile([128, 128], f16)              # kt0's lhsT
            fBh = big.tile([128, 128], f16)
            fA = big.tile([128, K_PAD], f16)
            fB = big.tile([128, K_PAD], f16)
            cRa = big.tile([128, AB_W], f16)
            cRb = big.tile([128, A1_W], f16)
            vt = big.tile([128, NKT * 65], f32)
            invx = big.tile([128, NKT], f32)
            beta = big.tile([128, 1], f32)
            wA = big.tile([128, NKT * MB2], bf16)
            rhsp = big.tile([128, NKT * 65], bf16)
            cvall = big.tile([128, NKT], f32)
            aggsb = big.tile([128, NCH * 65], f32)
            aggsbB = big.tile([128, NCH * 65], f32)
            filler = big.tile([128, 444], bf16)

            # ---- input DMAs (critical kt0/cR first on fast queues) ----
            nc.sync.dma_start(out=fAh[:], in_=fa_d[:, :128])
            nc.sync.dma_start(out=fBh[:], in_=fb_d[:, :128])
            nc.sync.dma_start(out=cRa[:], in_=cr_d[:, 0:AB_W])
            nc.sync.dma_start(out=cRb[:], in_=cr_d[:, AB_W:M_PAD])
            nc.gpsimd.dma_start(out=fA[:, 128:K_PAD],
                                in_=fa_d[:, 128:K_PAD])
            nc.gpsimd.dma_start(out=fB[:, 128:K_PAD],
                                in_=fb_d[:, 128:K_PAD])
            nc.scalar.dma_start(out=vt[:], in_=vt_d[:, :])
            nc.scalar.dma_start(out=invx[:], in_=invx_d[:, :])
            nc.scalar.dma_start(out=beta[:], in_=beta_d[:, :])

            # wA pad columns [2116:2176) per kt: one strided memset
            wA3 = wA[:].rearrange("p (t m) -> p t m", t=NKT)
            nc.vector.memset(wA3[:, :, M_PAD:MB2], 0.0)
            nc.vector.memset(filler[:], 0.0)

            with tc.tile_pool(name="sm", bufs=4) as smp, \
                 tc.tile_pool(name="azbs", bufs=2) as azbsp, \
                 tc.tile_pool(name="pa", bufs=2, space="PSUM") as pa, \
                 tc.tile_pool(name="pb", bufs=1, space="PSUM") as pb, \
                 tc.tile_pool(name="pca", bufs=1, space="PSUM") as pca, \
                 tc.tile_pool(name="pa1", bufs=1, space="PSUM") as pa1:

                # azA1 tile: az in [0:580), PE filler scratch in [580:1024)
                # (disjoint subtile ranges -> fillers dodge the WAR on the
                # single-buffered az region and keep the PE p-state ramp hot)
                a1t = pa1.tile([128, A1_BIG], f32, tag="a1")

                def fill(n):
                    for _ in range(n):
                        nc.tensor.matmul(out=a1t[:, A1_W:A1_BIG],
                                         lhsT=filler[:, 0:128],
                                         rhs=filler[:], start=True, stop=True)

                fill(NFILL)   # p-state ramp burn during the DMA lead-in

                acc1 = pca.tile([128, 512], f32, tag="acc1")
                b2a_sched = {9: (0, 4), 10: (4, 8), 11: (8, 11),
                             12: (11, 14), 13: (14, NCH)}
                b2a_drained = [0]

                def b2a_run(lo, hi):
                    for ci in range(lo, hi):
                        off = (ci % 7) * 65
                        for k2 in range(8):
                            w2 = k2 * MB2 + ci * 128
                            nc.tensor.matmul(
                                out=acc1[:, off:off + 65],
                                lhsT=wA[:, w2:w2 + 128],
                                rhs=rhsp[:, k2 * 65:(k2 + 1) * 65],
                                start=(k2 == 0), stop=(k2 == 7))
                        if ci % 7 == 6 or ci == NCH - 1:
                            lo_ci = ci - (ci % 7)
                            w = (ci % 7 + 1) * 65
                            nc.scalar.copy(
                                out=aggsb[:, lo_ci * 65:lo_ci * 65 + w],
                                in_=acc1[:, 0:w])

                for kt in range(NKT):
                    lA = fAh[:] if kt == 0 else fA[:, kt * 128:(kt + 1) * 128]
                    lB = fBh[:] if kt == 0 else fB[:, kt * 128:(kt + 1) * 128]

                    # ---- B1: azA, azB (double-buffered), 2 fp16 passes
                    aa = pa.tile([128, A_W], f32, tag="aa")
                    for lo, hi in ((0, 512), (512, A_W)):
                        nc.tensor.matmul(out=aa[:, lo:hi], lhsT=lA,
                                         rhs=cRa[:, lo:hi],
                                         start=True, stop=False)
                        nc.tensor.matmul(out=aa[:, lo:hi], lhsT=lB,
                                         rhs=cRa[:, lo:hi],
                                         start=False, stop=True)
                    bb = pb.tile([128, B_W], f32, tag="bb")
                    nc.tensor.matmul(out=bb[:], lhsT=lA,
                                     rhs=cRa[:, A_W:AB_W],
                                     start=True, stop=False)
                    nc.tensor.matmul(out=bb[:], lhsT=lB,
                                     rhs=cRa[:, A_W:AB_W],
                                     start=False, stop=True)
                    # fillers bridge the wait for azA1's WAR release
                    fill(NFILL_KT)
                    # ---- azA1 (single-buffered; the serialization window:
                    # its Sign is ordered first after rmax) ----
                    for lo, hi in ((0, 512), (512, A1_W)):
                        nc.tensor.matmul(out=a1t[:, lo:hi], lhsT=lA,
                                         rhs=cRb[:, lo:hi],
                                         start=True, stop=False)
                        nc.tensor.matmul(out=a1t[:, lo:hi], lhsT=lB,
                                         rhs=cRb[:, lo:hi],
                                         start=False, stop=True)

                    # ---- exact row max (DVE) + combine (Pool) ----
                    # azB: ACT copies PSUM->SBUF immediately (releases the
                    # single PSUM buffer early; Pool runs azB's one-hot off
                    # the copy, DVE reduces it)
                    azbs = azbsp.tile([128, B_W], f32, tag="azbs")
                    nc.scalar.copy(out=azbs[:], in_=bb[:])
                    maa = smp.tile([128, 1], f32, tag="maa")
                    nc.vector.reduce_max(out=maa[:], in_=aa[:], axis=X)
                    mbb = smp.tile([128, 1], f32, tag="mbb")
                    nc.vector.reduce_max(out=mbb[:], in_=azbs[:], axis=X)
                    mab = smp.tile([128, 1], f32, tag="mab")
                    nc.gpsimd.tensor_scalar(out=mab[:], in0=maa[:],
                                            scalar1=mbb[:], scalar2=None,
                                            op0=MAX)
                    ma1 = smp.tile([128, 1], f32, tag="ma1")
                    nc.vector.reduce_max(out=ma1[:], in_=a1t[:, 0:A1_W],
                                         axis=X)
                    wk = kt * MB2
                    rmax = smp.tile([128, 1], f32, tag="rmax")
                    nrm = smp.tile([128, 1], f32, tag="nrm")
                    nc.gpsimd.tensor_scalar(out=rmax[:], in0=mab[:],
                                            scalar1=ma1[:],
                                            scalar2=None, op0=MAX)
                    nc.gpsimd.tensor_scalar(out=nrm[:], in0=rmax[:],
                                            scalar1=-1.0,
                                            scalar2=None, op0=MULT)
                    nc.scalar.activation(out=wA[:, wk + AB_W:wk + M_PAD],
                                         in_=a1t[:, 0:A1_W], func=Sgn,
                                         bias=nrm[:])
                    nc.scalar.activation(out=cvall[:, kt:kt + 1],
                                         in_=rmax[:], func=Sig,
                                         bias=beta[:],
                                         scale=invx[:, kt:kt + 1])
                    nc.scalar.activation(out=wA[:, wk:wk + A_W],
                                         in_=aa[:], func=Sgn, bias=nrm[:])
                    nc.gpsimd.tensor_scalar(
                        out=wA[:, wk + A_W:wk + AB_W], in0=azbs[:],
                        scalar1=rmax[:], scalar2=-1.0, op0=IS_LT, op1=MULT)
                    nc.gpsimd.tensor_scalar(
                        out=rhsp[:, kt * 65:(kt + 1) * 65],
                        in0=vt[:, kt * 65:(kt + 1) * 65],
                        scalar1=cvall[:, kt:kt + 1], scalar2=None, op0=MULT)

                    if kt in b2a_sched:
                        b2a_run(*b2a_sched[kt])

            # ---- B2: transposed aggregation GEMM at the end (az pools
            # freed above; PE re-ramps through its first ~3us) ----
            with tc.tile_pool(name="pacc", bufs=1, space="PSUM") as paccp:
                # B2b: kt 8..13; 7-packed groups, 2 banks round-robin
                acc = paccp.tile([128, 1024], f32, tag="acc")
                for ci in range(NCH):
                    bank = (ci // 7) % 2
                    off = bank * 512 + (ci % 7) * 65
                    for k2 in range(8, NKT):
                        w2 = k2 * MB2 + ci * 128
                        nc.tensor.matmul(
                            out=acc[:, off:off + 65],
                            lhsT=wA[:, w2:w2 + 128],
                            rhs=rhsp[:, k2 * 65:(k2 + 1) * 65],
                            start=(k2 == 8), stop=(k2 == NKT - 1))
                    if ci % 7 == 6 or ci == NCH - 1:
                        lo_ci = ci - (ci % 7)
                        w = (ci % 7 + 1) * 65
                        if (ci // 7) % 2 == 0:
                            nc.vector.tensor_copy(
                                out=aggsbB[:, lo_ci * 65:lo_ci * 65 + w],
                                in_=acc[:, bank * 512:bank * 512 + w])
                        else:
                            nc.scalar.copy(
                                out=aggsbB[:, lo_ci * 65:lo_ci * 65 + w],
                                in_=acc[:, bank * 512:bank * 512 + w])
            nc.sync.dma_start(out=cv_d[:, :], in_=cvall[:])
            for lo, hi in ((0, 6 * 65), (6 * 65, 12 * 65),
                           (12 * 65, NCH * 65)):
                nc.sync.dma_start(out=agg_d[:, lo:hi], in_=aggsb[:, lo:hi])
                nc.scalar.dma_start(out=aggB_d[:, lo:hi],
                                    in_=aggsbB[:, lo:hi])
    nc.compile()
    return nc


def _f32(x):
    return np.ascontiguousarray(np.asarray(x), dtype=np.float32)


def _region_indices(points):
    rh = np.float32(SIZE_H / FOLD_H)
    rw = np.float32(SIZE_W / FOLD_W)
    px, py = points[:, 0], points[:, 1]
    idxs = []
    for i in range(FOLD_H):
        for j in range(FOLD_W):
            m = (py > rh * i) & (py <= rh * (i + 1)) & \
                (px > rw * j) & (px <= rw * (j + 1))
            idxs.append(np.nonzero(m)[0])
    return idxs


def _bilinear_taps(pts):
    one = np.float32(1.0)
    gridx = pts[:, 0] / np.float32(SIZE_W - 1.0) * np.float32(2.0) - one
    gridy = pts[:, 1] / np.float32(SIZE_H - 1.0) * np.float32(2.0) - one
    gx = (gridx + one) * np.float32(RW * 0.5) - np.float32(0.5)
    gy = (gridy + one) * np.float32(RH * 0.5) - np.float32(0.5)
    x0 = np.floor(gx)
    y0 = np.floor(gy)
    wx = (gx - x0).astype(np.float32)
    wy = (gy - y0).astype(np.float32)
    x0i = np.clip(x0, 0, RW - 1).astype(np.int32)
    x1i = np.clip(x0 + 1.0, 0, RW - 1).astype(np.int32)
    y0i = np.clip(y0, 0, RH - 1).astype(np.int32)
    y1i = np.clip(y0 + 1.0, 0, RH - 1).astype(np.int32)
    taps = np.stack([y0i * RW + x0i, y0i * RW + x1i,
                     y1i * RW + x0i, y1i * RW + x1i], axis=1)
    w = np.stack([(one - wx) * (one - wy), wx * (one - wy),
                  (one - wx) * wy, wx * wy], axis=1).astype(np.float32)
    # Clamp-collapsed points (all 4 taps at one pixel, e.g. ghost slots and
    # border points): weight (1,0,0,0) makes those columns bit-identical to
    # the ghost column, so argmax ties are exact and deterministic.
    collapsed = (x0i == x1i) & (y0i == y1i)
    w[collapsed] = np.array([1.0, 0.0, 0.0, 0.0], np.float32)
    return taps, w


def _hilo(x):
    hi = x.astype(np.float16)
    lo = (x - hi.astype(np.float32)).astype(np.float16)
    return hi, lo


def kernel(points, x, W_f, b_f, W_v, b_v, W_proj, b_proj, sim_alpha, sim_beta):
    from concourse.bass_utils import run_bass_kernel_spmd

    points = _f32(points)[0]
    x = _f32(x)[0]
    W_f, b_f = _f32(W_f), _f32(b_f)
    W_v, b_v = _f32(W_v), _f32(b_v)
    W_proj, b_proj = _f32(W_proj), _f32(b_proj)
    alpha = _f32(sim_alpha).reshape(-1)[0]
    beta = _f32(sim_beta).reshape(-1)[0]
    N = points.shape[0]

    idxs = _region_indices(points)
    cnts = [len(ix) for ix in idxs]
    assert max(cnts) + 1 <= M_PAD, cnts

    Wfb = np.concatenate([W_f.T, b_f[None, :]], axis=0).astype(np.float32)
    Wvb = np.concatenate([W_v.T, b_v[None, :]], axis=0).astype(np.float32)
    beta128 = np.full((128, 1), beta, np.float32)

    in_maps = []
    vcts = []
    vts_host = []
    for r in range(R):
        i, j = divmod(r, FOLD_W)
        xr = x[:, i * RH:(i + 1) * RH, j * RW:(j + 1) * RW].reshape(64, HW)
        idx_r = idxs[r]
        cnt = len(idx_r)
        pts_r = np.zeros((cnt + 1, 2), np.float32)
        pts_r[:cnt] = points[idx_r]
        taps, w = _bilinear_taps(pts_r)
        g = xr[:, taps]                                    # [64, cnt+1, 4]
        xg = np.einsum("cmt,mt->cm", g, w).astype(np.float32)
        xg1 = np.ascontiguousarray(
            np.concatenate([xg, np.ones((1, cnt + 1), np.float32)], axis=0))
        # centers + l2 scale (alpha folded in) -> scaled center features
        centers = (xg1.T @ Wfb).astype(np.float32)         # [cnt+1, 64]
        nc2 = (centers * centers).sum(axis=1, dtype=np.float32)
        s = ((np.float32(1.0) / np.sqrt(nc2 + np.float32(1e-12))) * alpha
             ).astype(np.float32)
        cnhatT = (centers * s[:, None]).T                  # [64, cnt+1]
        chi, clo = _hilo(cnhatT)
        cR = np.empty((128, M_PAD), np.float16)
        cR[0:64, :cnt + 1] = chi
        cR[64:128, :cnt + 1] = clo
        cR[0:64, cnt + 1:] = chi[:, cnt:cnt + 1]           # ghost copies
        cR[64:128, cnt + 1:] = clo[:, cnt:cnt + 1]
        # value centers (host side of the output combine)
        vcT = np.ascontiguousarray((xg1.T @ Wvb).T)        # [64, cnt+1]
        vcts.append(vcT)
        # full-map features for sim columns + alpha/|feat| scales
        xr1 = np.concatenate([xr, np.ones((1, HW), np.float32)], axis=0)
        featT = (xr1.T @ Wfb).astype(np.float32)           # [HW, 64]
        nfx = (featT * featT).sum(axis=1, dtype=np.float32)
        invx_full = (np.float32(1.0) / np.sqrt(nfx + np.float32(1e-12))
                     ).astype(np.float32)
        # per-column values (incl bias row + kmask col)
        vt_full = (xr1.T @ np.concatenate(
            [Wvb, np.zeros((65, 1), np.float32)], axis=1)).astype(np.float32)
        vt_full[:, 64] = 1.0                               # kmask for real k
        for h in range(2):
            fh = np.zeros((64, K_PAD), np.float32)
            fh[:, :K_HALF] = featT[h * K_HALF:(h + 1) * K_HALF].T
            fhi, flo = _hilo(fh)
            z = np.zeros_like(fhi)
            fAm = np.ascontiguousarray(np.concatenate([fhi, z], axis=0))
            fBm = np.ascontiguousarray(np.concatenate([flo, fhi], axis=0))
            vt_np = np.zeros((K_PAD, 65), np.float32)
            vt_np[:K_HALF] = vt_full[h * K_HALF:(h + 1) * K_HALF, :65]
            vt_in = np.ascontiguousarray(
                vt_np.reshape(NKT, 128, 65).transpose(1, 0, 2).reshape(
                    128, NKT * 65))
            iv = np.ones((K_PAD,), np.float32)
            iv[:K_HALF] = invx_full[h * K_HALF:(h + 1) * K_HALF]
            invx = np.ascontiguousarray(iv.reshape(NKT, 128).T)  # [128, NKT]
            vts_host.append(vt_np)
            in_maps.append({
                "fA": fAm, "fB": fBm, "cR": cR, "vt": vt_in,
                "invx": invx, "beta128": beta128,
            })

    global _LAST_IN_MAPS
    _LAST_IN_MAPS = in_maps
    if 0 not in _BUILT:
        _BUILT[0] = _build()
    res = run_bass_kernel_spmd(_BUILT[0], in_maps,
                               core_ids=list(range(N_CORES)))
    results = res.results

    out = np.zeros((64, N), np.float32)
    for r in range(R):
        agg = None
        rs = np.zeros((65,), np.float32)
        for h in range(2):
            core = 2 * r + h
            rr = results[core]
            # agg[m, c] with m = ci*128 + p
            a = (rr["agg"] + rr["aggB"]).reshape(
                128, NCH, 65).transpose(1, 0, 2).reshape(NCH * 128, 65)
            agg = a if agg is None else agg + a
            # rs[c] = sum_k rhsp[k, c] with the device's bf16 rounding
            # (B2's -1 offset uses bf16 rhsp, so the correction must too)
            cv = rr["cv"].T.reshape(K_PAD)           # [NKT,128] -> k order
            rhs_bf = (vts_host[core] * cv[:, None]).astype(
                ml_dtypes.bfloat16).astype(np.float32)
            rs += rhs_bf.sum(axis=0)
        idx_r = idxs[r]
        cnt = len(idx_r)
        vcT = vcts[r]
        a = agg[:cnt] + rs[None, :]
        ort = (a[:, :64].T + vcT[:, :cnt]) / \
            (a[:, 64] + np.float32(1.0))[None, :]
        proj = W_proj @ ort + b_proj[:, None]
        mask = np.any(ort != 0.0, axis=0)
        out[:, idx_r] = proj * mask[None, :]
    return out[None, :, None, :]
